# revision 49
# baseline (speedup 1.0000x reference)
"""Multi-head causal attention (B=1, T=4096, D=768, H=12) on 8 trn2 cores.

Sharding: every core runs the IDENTICAL program (SPMD) with two head
slots; cores differ only in weight/input data.
  slot A (partitions 0:64):  one whole head (heads 0..7 on cores 0..7)
  slot B (partitions 64:128): HALF of a split head.  Heads 8..11 are each
    split into even/odd key-chunk interleaves; core 2p gets the even
    chunks of head 8+p, core 2p+1 the odd chunks.  The program always
    processes "key chunk j" of a packed 2048-token stream xB whose
    chunks the HOST packed as x chunks 2j+delta — so the even/odd choice
    is pure data.  Per-core mask tables supply the two diagonal-straddle
    patterns ({0,2} for even, {1,3} for odd).
Per-core causal work: 144 (whole) + 72 (half) = 216 key chunks = the
ideal 12*144/8 balance; no dummy compute anywhere.

Score matmuls have contraction K=64 (head dim), i.e. they occupy only
half of the 128-row PE array.  This version row-tiles them in pairs:
the PE runs two K=64 matmuls CONCURRENTLY when they sit at different
row offsets (tile_position auto-derives from the operand base
partition).  Every score group is one such pair:
  mixed pairs (A chunk @ partitions 0:64, B chunk @ 64:128) — both
    operands in their natural positions, no extra data movement;
  AA' pairs (A chunk @ 0:64, A chunk kc%4==3 @ 64:128) — the upper
    operands (Q_A and the kc%4==3 K_A chunks) are duplicated to
    partitions 64:128 by SBUF->SBUF DMAs right after each projection
    token tile.  AA' pairs close out each query tile so the dup DMAs
    are never on the critical path.
The out-projection matmuls (also K=64) are row-tiled the same way:
slot A (wo rows 0:64) and slot B (rows 64:128) issue back-to-back into
one [128,1024] PSUM tile and are evacuated by a single DVE cast.

The kernel emits unnormalized per-slot out-projections plus the softmax
denominators (replicated into a 64-row PSUM block by 64 ones-columns in
the AV lhsT); the host performs the flash-attention-style combination
  out = sum_c O_A_c/den_A_c + sum_p (O_B_2p + O_B_2p+1)/(den_B_2p + den_B_2p+1)

Pipeline: scores are issued with lookahead ahead of exp/AV; projection,
V-transpose and out-projection units are deferred and dripped one per
score group so the ACT exp stream (the throughput floor) never starves.
"""

import math
import numpy as np
import ml_dtypes
from contextlib import ExitStack
from collections import deque

import concourse.bass as bass
import concourse.bacc as bacc
import concourse.mybir as mybir
import concourse.tile as tile
from concourse.bass_utils import run_bass_kernel_spmd

BF16 = mybir.dt.bfloat16
F32 = mybir.dt.float32
AF = mybir.ActivationFunctionType

T = 4096
TB = 2048                 # packed token stream length for the half slot
D_MODEL = 768
HEAD_DIM = 64
N_HEADS = 12
N_CORES = 8
QT = 512                  # query tile width
KC = 128                  # key chunk (psum partition dim)
NPAT = 4                  # straddle mask patterns per query tile (QT//KC)
NQT = T // QT             # 8 query tiles
CCH = D_MODEL // 128      # 6 contraction chunks
TOKT = 512                # token tile for projections
NTOKT = T // TOKT
NTOKB = TB // TOKT

_PROGRAM_CACHE = {}


def build_program():
    nc = bacc.Bacc(None)

    # x inputs are HOST-PREPARED in token-tile-major SBUF layout
    # [128, tt, cch, tok'] so every token-tile DMA is a 2D contiguous slab
    # (6 KB per partition row -> large descriptors, near-peak HBM BW,
    # cheap triggers).  Same for the merged output (see outAB below).
    xT_d = nc.declare_dram_parameter("xT", [128, CCH * T], BF16, isOutput=False)
    xB_d = nc.declare_dram_parameter("xB", [128, CCH * TB], BF16, isOutput=False)
    # weights pre-arranged by host into SBUF layout:
    # wqkv[128, k] columns: s-major (q,k,v), then cch chunk, then 128 head dims
    w_d = nc.declare_dram_parameter("wqkv", [128, 3 * CCH * 128], BF16, isOutput=False)
    b_d = nc.declare_dram_parameter("bqkv", [128, 3], F32, isOutput=False)
    wo_d = nc.declare_dram_parameter("wo2", [128, D_MODEL], BF16, isOutput=False)
    # masks: [128, 6*QT]: 4 slot-A straddle patterns then 2 slot-B patterns
    mk_d = nc.declare_dram_parameter("masks", [128, 6 * QT], BF16, isOutput=False)
    id_d = nc.declare_dram_parameter("ident", [128, 128], BF16, isOutput=False)
    # merged output: per query tile one contiguous [128, CCH*2*QT] slab
    # laid out [dch][slotA q | slotB q]; host unpacks
    outAB_d = nc.declare_dram_parameter(
        "outAB", [128, NQT * CCH * 2 * QT], BF16, isOutput=True)
    denA_d = nc.declare_dram_parameter("denA", [1, T], BF16, isOutput=True)
    denB_d = nc.declare_dram_parameter("denB", [1, T], BF16, isOutput=True)

    with tile.TileContext(nc) as tc, ExitStack() as ctx:
        consts = ctx.enter_context(tc.tile_pool(name="consts", bufs=1))
        big = ctx.enter_context(tc.tile_pool(name="big", bufs=1))
        ptp = ctx.enter_context(tc.tile_pool(name="ptp", bufs=8))
        hvp = ctx.enter_context(tc.tile_pool(name="hvp", bufs=6))
        osb = ctx.enter_context(tc.tile_pool(name="osb", bufs=2))
        # PSUM: score/proj/outproj pool 2 banks x3 bufs, av 2 banks x1 = 8
        sp = ctx.enter_context(tc.tile_pool(name="sp", bufs=3, space="PSUM"))
        avpA = ctx.enter_context(tc.tile_pool(name="avpA", bufs=1, space="PSUM"))
        avpB = ctx.enter_context(tc.tile_pool(name="avpB", bufs=1, space="PSUM"))

        # ---- constants to SBUF ----
        # w/b gate the prologue on the gpsimd SWDGE queue; id rides the
        # sync HW queue first (tiny) so warmup transposes can start
        # immediately; masks/wo follow the first xB tile.
        id_sb = consts.tile([128, 128], BF16, tag="id")
        nc.sync.dma_start(out=id_sb[:], in_=id_d[:, :])
        w_sb = consts.tile([128, 3 * CCH * 128], BF16, tag="w")
        nc.gpsimd.dma_start(out=w_sb[:], in_=w_d[:, :])
        b_sb = consts.tile([128, 3], F32, tag="b")
        nc.gpsimd.dma_start(out=b_sb[:], in_=b_d[:, :])
        wo_sb = consts.tile([128, D_MODEL], BF16, tag="wo")
        mask_sb = consts.tile([128, 6 * QT], BF16, tag="mask")
        maskA = mask_sb[:, 0:NPAT * QT]
        maskB = mask_sb[:, NPAT * QT:6 * QT]

        # ---- x inputs: one contiguous 2D slab per token tile ----
        # host layout [128, tt, cch, tok']; xs(j, tt) addresses one
        # projection operand chunk.
        xTall = big.tile([128, CCH * T], BF16, tag="xTall")
        xBall = big.tile([128, CCH * TB], BF16, tag="xBall")
        TSLAB = CCH * TOKT

        def xs(j, tt, half):
            a = xBall if half else xTall
            o = (tt * CCH + j) * TOKT
            return a[:, o:o + TOKT]

        for tt in range(NTOKT):
            sl = slice(tt * TSLAB, (tt + 1) * TSLAB)
            if tt == 0:
                # split tile 0 across both HW rings for minimum latency
                h = TSLAB // 2
                nc.scalar.dma_start(out=xTall[:, 0:h], in_=xT_d[:, 0:h])
                nc.sync.dma_start(out=xTall[:, h:TSLAB], in_=xT_d[:, h:TSLAB])
            else:
                nc.scalar.dma_start(out=xTall[:, sl], in_=xT_d[:, sl])
            if tt < NTOKB:
                nc.gpsimd.dma_start(out=xBall[:, sl], in_=xB_d[:, sl])
            if tt == 0:
                # sync HW ring: keeps the SWDGE ring clear so xB1 arrives
                # before proj-pair2-tt1 needs it (the dup DMAs these delay
                # are not consumed until query tile 2)
                nc.sync.dma_start(out=mask_sb[:], in_=mk_d[:, :])
                nc.sync.dma_start(out=wo_sb[:], in_=wo_d[:, :])

        # HAM warmup + ACT table preload during the input-DMA wait: the
        # transposes depend on progressively-arriving data (id, then each
        # half of x token tile 0) so the PE ticks over instead of idling.
        # the narrowed straddle mask/AV never read the pt prefix that the
        # narrowed exp skips, so no pt-slot initialization is needed
        dummy = ptp.tile([128, 2 * QT], BF16, tag="pt", name="warm")
        nc.scalar.activation(dummy[:, 0:1], id_sb[:, 0:1], AF.Exp)
        for src in (id_sb[:, 0:128], xTall[:, 0:128],
                    xTall[:, TSLAB // 2:TSLAB // 2 + 128]):
            wt = sp.tile([128, 512], BF16, tag="sc", name="warm")
            for i in range(4):
                nc.tensor.transpose(wt[:, i * 128:(i + 1) * 128], src,
                                    id_sb[:])

        # ---- projections ----
        # QT_sb: merged A+B queries (same tokens).  KT_sb/VT_sb: partitions
        # 0:64 = slot A over x, partitions 64:128 = slot B over packed xB.
        QT_sb = big.tile([128, T], BF16, tag="Q")
        KT_sb = big.tile([128, T], BF16, tag="K")
        VT_sb = big.tile([128, T], BF16, tag="VT")
        # partition-swapped dup copies for the A-A' row-tiled score pairs:
        # QX partitions 64:128 = Q_A; KX partitions 64:128 = K_A chunks
        # with kc%4==3 (slot kc//4).  B jobs are naturally high, so mixed
        # (A-low, B-high) pairs need no dups at all.
        QX_sb = big.tile([128, T], BF16, tag="QX")
        KX_sb = big.tile([128, T // 4], BF16, tag="KX")
        # V2A per key chunk c: [V_A | ones64] -> AV rows 0:64, den x64 rows 64:128
        # V2B per key chunk c: [ones64 | V_B] -> den x64 rows 0:64, AV rows 64:128
        V2A = big.tile([128, (T // 128) * 128], BF16, tag="V2A")
        V2B = big.tile([128, (TB // 128) * 128], BF16, tag="V2B")
        va3 = V2A[:].rearrange("p (t c) -> p t c", c=128)
        nc.vector.memset(va3[:, :, 64:128], 1.0)
        vb3 = V2B[:].rearrange("p (t c) -> p t c", c=128)
        nc.vector.memset(vb3[:, :, 0:64], 1.0)

        def proj_unit(s, tt, half):
            # s=0 (Q): merged A+B (m=128) over x.
            # s=1, half=False ("pair1"): lhsT [wk_A | wv_A] over x ->
            #   psum rows 0:64 = K_A, rows 64:128 = V_A.
            # s=1, half=True ("pair2"): lhsT [wv_B | wk_B] over xB ->
            #   psum rows 0:64 = V_B, rows 64:128 = K_B.
            # (host pre-arranged w_sb columns to match; VT_sb holds V_A at
            # partitions 64:128 and V_B at 0:64.)
            def emit():
                pp = sp.tile([128, TOKT], F32, tag="sc", name="pp")
                wblk = 2 if half else s
                for j in range(CCH):
                    base = (wblk * CCH + j) * 128
                    nc.tensor.matmul(
                        pp[:], w_sb[:, base:base + 128],
                        xs(j, tt, half),
                        start=(j == 0), stop=(j == CCH - 1),
                    )
                sl = slice(tt * TOKT, (tt + 1) * TOKT)
                if s == 0:
                    nc.vector.tensor_scalar_add(
                        QT_sb[:, sl], pp[:], b_sb[:, 0:1])
                    # dup: Q_A -> QX upper
                    nc.sync.dma_start(out=QX_sb[64:128, sl],
                                      in_=QT_sb[0:64, sl])
                elif not half:
                    nc.vector.tensor_scalar_add(
                        KT_sb[0:64, sl], pp[0:64, :], b_sb[0:64, 1:2])
                    nc.vector.tensor_scalar_add(
                        VT_sb[64:128, sl], pp[64:128, :], b_sb[64:128, 1:2])
                    # dup: K_A chunk 4tt+3 -> KX upper slot tt
                    nc.sync.dma_start(
                        out=KX_sb[64:128, tt * 128:(tt + 1) * 128],
                        in_=KT_sb[0:64, (4 * tt + 3) * 128:(4 * tt + 4) * 128])
                else:
                    nc.vector.tensor_scalar_add(
                        VT_sb[0:64, sl], pp[0:64, :], b_sb[0:64, 2:3])
                    nc.vector.tensor_scalar_add(
                        KT_sb[64:128, sl], pp[64:128, :], b_sb[64:128, 2:3])
            return emit

        def tps_unit(tt):
            # 4 transposes of one token tile share a single PSUM slot
            # ([128,512] bf16 fits one bank); V2A/V2B fills are merged
            # strided DVE copies.
            def emit():
                c0 = tt * TOKT // 128
                tp = sp.tile([128, 512], BF16, tag="sc", name="tp")
                for i in range(4):
                    c = c0 + i
                    nc.tensor.transpose(
                        tp[:, i * 128:(i + 1) * 128],
                        VT_sb[:, c * 128:(c + 1) * 128], id_sb[:])
                tp3 = tp[:].rearrange("p (i c) -> p i c", c=128)
                vaw = va3[:, c0:c0 + 4, :]
                nc.vector.tensor_copy(vaw[:, :, 0:64], tp3[:, :, 64:128])
                if c0 < TB // 128:
                    vbw = vb3[:, c0:c0 + 4, :]
                    nc.vector.tensor_copy(vbw[:, :, 64:128], tp3[:, :, 0:64])
            return emit

        def units_for(tt):
            u = [proj_unit(0, tt, False), proj_unit(1, tt, False)]
            if tt < NTOKB:
                u.append(proj_unit(1, tt, True))
            u.append(tps_unit(tt))
            return u

        av_tiles = {}
        projq = []
        deferred = []

        def score_ops(kc, h, qs, qi):
            # returns (lhsT, rhs) for score job (kc, h); slot-A chunks with
            # kc%4==3 run on the upper array half from the dup tiles.  In
            # the first two query tiles the dup DMAs have little slack and
            # a wait would stall the in-order PE queue, so those jobs run
            # serially from the natural low position instead.
            if h == 0:
                if kc % 4 != 3 or qi < 2:
                    return (KT_sb[0:64, kc * KC:(kc + 1) * KC],
                            QT_sb[0:64, qs:qs + QT])
                s0 = (kc // 4) * 128
                return (KX_sb[64:128, s0:s0 + 128],
                        QX_sb[64:128, qs:qs + QT])
            return (KT_sb[64:128, kc * KC:(kc + 1) * KC],
                    QT_sb[64:128, qs:qs + QT])

        def issue_scores(G):
            qi, grp, first, last = G
            qs = qi * QT
            sc = sp.tile([128, 2 * QT], F32, tag="sc")
            for ji, (kc, h, fi, la) in enumerate(grp):
                lhsT, rhs = score_ops(kc, h, qs, qi)
                nc.tensor.matmul(
                    sc[:, ji * QT:(ji + 1) * QT],
                    lhsT, rhs,
                    start=True, stop=True,
                )
            return sc

        def issue_rest(G, sc):
            qi, grp, first, last = G
            last_qi = qi == NQT - 1
            qs = qi * QT
            nstepA = (qi + 1) * QT // KC
            nstepB = nstepA // 2
            if first:
                av_tiles[qi] = (avpA.tile([128, QT], F32, tag="avA", name="avA"),
                                avpB.tile([128, QT], F32, tag="avB", name="avB"))
            avA, avB = av_tiles[qi]
            pt = ptp.tile([128, 2 * QT], BF16, tag="pt")
            # the (A pat2, A' pat3) straddle group's first 256 queries are
            # fully masked; skip them in the exp (the mask-mul zeroes the
            # stale region afterwards)
            e0 = 0
            if (grp[0][1] == 0 and grp[1][1] == 0
                    and grp[0][0] == nstepA - 2 and grp[1][0] == nstepA - 1):
                e0 = QT // 2
            elif grp[0] == [nstepA - 3, 0, False, False] and grp[1][1] == 1:
                # second mixed-straddle group: slot A pattern 1's first
                # 128 queries are fully masked
                e0 = QT // 4
            nc.scalar.activation(
                pt[:, e0:2 * QT], sc[:, e0:2 * QT], AF.Exp,
                scale=1.0 / math.sqrt(HEAD_DIM),
            )
            for ji, (kc, h, fi, la) in enumerate(grp):
                nstep = nstepB if h else nstepA
                npat = NPAT // 2 if h else NPAT
                mtab = maskB if h else maskA
                # queries below q0 are fully masked for straddle patterns:
                # narrow the mask-mul and the AV stream to [q0:QT].  For
                # slot B the per-core delta shifts the true threshold, so
                # use the conservative (delta=0) width.  fi jobs are
                # always pattern 0 (q0=0), keeping psum-clear semantics.
                q0 = 0
                if kc >= nstep - npat:  # diagonal straddle
                    pat = kc - (nstep - npat)
                    q0 = 128 * (2 * pat if h else pat)
                    nc.vector.tensor_mul(
                        pt[:, ji * QT + q0:(ji + 1) * QT],
                        pt[:, ji * QT + q0:(ji + 1) * QT],
                        mtab[:, pat * QT + q0:(pat + 1) * QT])
                v2 = V2B if h else V2A
                nc.tensor.matmul(
                    (avB if h else avA)[:, q0:QT],
                    v2[:, kc * 128:(kc + 1) * 128],
                    pt[:, ji * QT + q0:(ji + 1) * QT],
                    start=fi, stop=la,
                )
            if projq:
                projq.pop(0)()
            elif deferred:
                deferred.pop(0)()
                if (last_qi or len(deferred) > 8) and deferred:
                    deferred.pop(0)()
            if not last:
                return
            # unnormalized per-slot out-projection; host divides by the
            # denominators (flash-attention-style partial combination).
            # The 6 paired (matmul+matmul+cast+dma) units are deferred and
            # spread one per subsequent score group.
            avA, avB = av_tiles.pop(qi)
            hvA = hvp.tile([128, QT], BF16, tag="hvA", name="hvA")
            nc.vector.tensor_copy(hvA[:], avA[:])
            hvB = hvp.tile([128, QT], BF16, tag="hvB", name="hvB")
            nc.vector.tensor_copy(hvB[:], avB[:])
            nc.sync.dma_start(out=denA_d[0:1, qs:qs + QT], in_=hvA[64:65, :])
            nc.sync.dma_start(out=denB_d[0:1, qs:qs + QT], in_=hvB[0:1, :])

            def mk_op(dch, hvA, hvB, ost, qs=qs):
                # row-tiled pair: slot A (wo rows 0:64, tile row 0) and
                # slot B (rows 64:128, tile row 64) run concurrently into
                # adjacent PSUM banks; one merged DVE cast evacuates both.
                def emit():
                    op = sp.tile([128, 2 * QT], F32, tag="sc", name="op")
                    nc.tensor.matmul(
                        op[:, 0:QT],
                        wo_sb[0:64, dch * 128:(dch + 1) * 128], hvA[0:64, :],
                        start=True, stop=True,
                    )
                    nc.tensor.matmul(
                        op[:, QT:2 * QT],
                        wo_sb[64:128, dch * 128:(dch + 1) * 128],
                        hvB[64:128, :],
                        start=True, stop=True,
                    )
                    if last_qi and dch % 2 == 1:
                        nc.scalar.activation(
                            ost[:, dch * 2 * QT:(dch + 1) * 2 * QT], op[:],
                            AF.Copy)
                    else:
                        nc.vector.tensor_copy(
                            ost[:, dch * 2 * QT:(dch + 1) * 2 * QT], op[:])
                    OSL = CCH * 2 * QT
                    if last_qi:
                        # tail: ship each dch pair as soon as it is cast,
                        # alternating queues, so the final DMA is tiny
                        eng = nc.gpsimd if dch % 2 else nc.sync
                        o0 = qi * OSL + dch * 2 * QT
                        eng.dma_start(
                            out=outAB_d[0:128, o0:o0 + 2 * QT],
                            in_=ost[:, dch * 2 * QT:(dch + 1) * 2 * QT])
                    elif dch == CCH - 1:
                        # one contiguous 2D slab DMA for the whole qtile
                        nc.sync.dma_start(
                            out=outAB_d[0:128, qi * OSL:(qi + 1) * OSL],
                            in_=ost[:])
                return emit
            ost = osb.tile([128, CCH * 2 * QT], BF16, tag="ot", name="ost")
            for dch in range(CCH):
                deferred.append(mk_op(dch, hvA, hvB, ost))

        # prologue: everything needed by query tile 0
        for u in units_for(0):
            u()

        pend = deque()
        for qi in range(NQT):
            while projq:   # units for tok tiles <= qi must be issued by now
                projq.pop(0)()
            if qi + 1 < NTOKT:
                projq.extend(units_for(qi + 1))
            nstepA = (qi + 1) * QT // KC
            nstepB = nstepA // 2
            # row-tiled pairs (low partition job, high partition job).
            # Mixed (A-low, B-high) pairs have no dup dependency and run
            # first (diagonal-straddle pairs leading, for mask slack);
            # the dup-gated (A-low, A'-high) pairs close out the qtile.
            n = qi + 1
            body_lowA = [kc for kc in range(nstepA - 4) if kc % 4 != 3]
            order = ([((nstepA - 4, 0), (nstepB - 2, 1)),
                      ((nstepA - 3, 0), (nstepB - 1, 1))]
                     + [((kc, 0), (j, 1)) for kc, j in
                        zip(body_lowA, range(nstepB - 2))]
                     + [((nstepA - 2, 0), (nstepA - 1, 0))]
                     + [((kc, 0), (kh, 0)) for kc, kh in
                        zip(body_lowA[2 * n - 2:],
                            [k for k in range(nstepA - 4) if k % 4 == 3])])
            seen = {}
            jobs = []
            for pr in order:
                for kc, h in pr:
                    jobs.append([kc, h, h not in seen, False])
                    seen[h] = len(jobs) - 1
            jobs[seen[0]][3] = True
            jobs[seen[1]][3] = True
            groups = [(qi, jobs[g0:g0 + 2], g0 == 0, g0 + 2 >= len(jobs))
                      for g0 in range(0, len(jobs), 2)]
            for G in groups:
                sc = issue_scores(G)
                pend.append((G, sc))
                if len(pend) > 3:
                    issue_rest(*pend.popleft())
        while pend:
            issue_rest(*pend.popleft())
        while projq:
            projq.pop(0)()
        while deferred:
            deferred.pop(0)()
    nc.finalize()
    return nc


def _host_inputs(x, wq, bq, wk, bk, wv, bv, wo):
    """Per-core input maps.  Slot A of core c = whole head c.  Slot B of
    core 2p+delta = the delta-interleave (even/odd key chunks) of head 8+p."""
    bf16 = ml_dtypes.bfloat16

    def ttmajor(a, t):
        # [768, t] -> token-tile-major [128, t//TOKT, CCH, TOKT]
        return np.ascontiguousarray(
            a.reshape(CCH, 128, t // TOKT, TOKT).transpose(1, 2, 0, 3)
        ).reshape(128, CCH * t)

    x0 = np.ascontiguousarray(x[0].T)          # [768, T]
    xT = ttmajor(x0, T).astype(bf16)
    # packed interleave streams: chunk j of xB[delta] = x chunk 2j+delta
    xc = x0.reshape(D_MODEL, T // 128, 128)
    xB = {d: ttmajor(np.ascontiguousarray(
        xc[:, d::2, :].reshape(D_MODEL, TB)), TB).astype(bf16)
        for d in (0, 1)}

    dk = np.arange(128)[:, None]
    dq = np.arange(QT)[None, :]
    ident = np.eye(128, dtype=np.float32).astype(bf16)

    in_maps = []
    for c in range(N_CORES):
        hA = c
        p, delta = divmod(c, 2)
        hB = 8 + p
        # weights pre-arranged into the on-chip SBUF layout
        # w_sb blocks: s=0 -> [wq_A | wq_B]; s=1 -> [wk_A | wv_A];
        #              s=2 -> [wv_B | wk_B]
        w = np.zeros((128, 3 * CCH * 128), np.float32)
        b = np.zeros((128, 3), np.float32)
        for j in range(CCH):
            rows = slice(j * 128, (j + 1) * 128)
            b0 = j * 128
            w[:, b0:b0 + 64] = wq[hA][rows, :]
            w[:, b0 + 64:b0 + 128] = wq[hB][rows, :]
            b1 = (CCH + j) * 128
            w[:, b1:b1 + 64] = wk[hA][rows, :]
            w[:, b1 + 64:b1 + 128] = wv[hA][rows, :]
            b2 = (2 * CCH + j) * 128
            w[:, b2:b2 + 64] = wv[hB][rows, :]
            w[:, b2 + 64:b2 + 128] = wk[hB][rows, :]
        b[0:64, 0] = bq[hA]
        b[64:128, 0] = bq[hB]
        b[0:64, 1] = bk[hA]
        b[64:128, 1] = bv[hA]
        b[0:64, 2] = bv[hB]
        b[64:128, 2] = bk[hB]
        wo2 = np.zeros((128, D_MODEL), np.float32)
        wo2[0:64, :] = wo[hA * 64:(hA + 1) * 64, :]
        wo2[64:128, :] = wo[hB * 64:(hB + 1) * 64, :]
        # masks: 4 slot-A patterns (0..3), then 2 slot-B patterns
        masks = np.zeros((128, 6 * QT), np.float32)
        for pat in range(4):
            masks[:, pat * QT:(pat + 1) * QT] = (dk + 128 * pat <= dq)
        for i, pat in enumerate((delta, delta + 2)):
            masks[:, (4 + i) * QT:(5 + i) * QT] = (dk + 128 * pat <= dq)
        in_maps.append({
            "xT": xT,
            "xB": xB[delta],
            "wqkv": w.astype(bf16),
            "bqkv": b.astype(np.float32),
            "wo2": wo2.astype(bf16),
            "masks": masks.astype(bf16),
            "ident": ident,
        })
    return in_maps


def kernel(_trace=False, _tmpdir=None, **inputs):
    x = np.asarray(inputs["x"], np.float32)
    args = (x,
            np.asarray(inputs["wq"], np.float32), np.asarray(inputs["bq"], np.float32),
            np.asarray(inputs["wk"], np.float32), np.asarray(inputs["bk"], np.float32),
            np.asarray(inputs["wv"], np.float32), np.asarray(inputs["bv"], np.float32),
            np.asarray(inputs["wo"], np.float32))
    bo = np.asarray(inputs["bo"], np.float32)

    if "nc" not in _PROGRAM_CACHE:
        _PROGRAM_CACHE["nc"] = build_program()
    nc = _PROGRAM_CACHE["nc"]

    in_maps = _host_inputs(*args)
    res = run_bass_kernel_spmd(
        nc, in_maps, list(range(N_CORES)), trace=_trace, tmpdir=_tmpdir,
    )
    def unpack(r):
        # [128, NQT*CCH*2*QT] -> (outA, outB) each [768, T]
        v = np.asarray(r["outAB"], np.float32).reshape(128, NQT, CCH, 2, QT)
        outs = []
        for slot in (0, 1):
            outs.append(np.transpose(v[:, :, :, slot, :], (2, 0, 1, 3))
                        .reshape(D_MODEL, T))
        return outs

    acc = np.zeros((D_MODEL, T), np.float32)
    oAB = [unpack(res.results[c]) for c in range(N_CORES)]
    for c in range(N_CORES):
        acc += oAB[c][0] / np.asarray(
            res.results[c]["denA"], np.float32)
    for p in range(4):
        rA, rB = res.results[2 * p], res.results[2 * p + 1]
        num = oAB[2 * p][1] + oAB[2 * p + 1][1]
        den = (np.asarray(rA["denB"], np.float32)
               + np.asarray(rB["denB"], np.float32))
        acc += num / den
    out = acc.T + bo[None, :]
    if _trace:
        return out[None].astype(np.float32), res
    return out[None].astype(np.float32)
